# revision 44
# baseline (speedup 1.0000x reference)
"""Trainium2 Bass kernel for AttentiveNonLocalBlock2D.

Pipeline (per core, SPMD over 8 NeuronCores, sequence-parallel over N=H*W):
  Phase A (replicated): 3x stride-2 conv gating unit (fp16 PE, lrelu as one
    DVE max-op) -> bilinear x8 upsample (DVE+Pool) -> sigmoid gate (ACT) ->
    xg16 = sigmoid(yh)*x in fp16.
  Algebraic fusion: f[n,m] = theta(:,n).phi(:,m) = sum_c v[c,n] xg[c,m] with
    v = Q xg, Q = Tw^T Pw (64x64). So no phi/theta projections: pass 1 uses
    xg m-tiles as the stationary operand (K=64) and v (own n-chunk only) as
    the moving operand. G^T tiles = (E^T xg)^T with E^T = gw^T WwT, all fp16.
  Pass 1: fps[m_tile, n_chunk] = xg_tile^T v_chunk (fp16 PE), exp(f - 5) via
    ACT into an SBUF fp16 cache; softmax denominators Z[m] via ACT accum_out
    (half) / DVE tensor_scalar accum (half). Z AllReduced in two halves.
  Pass 2: s-stationary matmuls: y2T[n_sub, c] += s_tile[128m x 128n]^T G_tile
    [128m x 64c] accumulated in PSUM across all 72 m-tiles; PSUM initialized
    with xg^T (PE transpose) so the residual is free. Host de-transposes.
"""

import sys

if "/opt/trn_rl_repo" not in sys.path:
    sys.path.insert(0, "/opt/trn_rl_repo")

import numpy as np

NCORES = 8
C, CI, H, W = 64, 32, 96, 96
N = H * W            # 9216
CH = N // NCORES     # 1152 pixels per core
MT = N // 128        # 72 m-tiles
HALF = MT // 2
KA = 36              # m-tiles covered by the first (early) Z AllReduce
# Schraudolph fp16 exp: bitcast(i16) with i16 = round(alpha*x + beta) approx
# exp(x - 5); alpha = 2^10/ln2, beta = 15360 - 59.33 (mean-centering) -
# 5*alpha (the -5 softmax guard bias).
EXP_ALPHA = 1477.3195
EXP_BETA = 15360.0 - 59.33 - 5.0 * EXP_ALPHA
NSUB = CH // 128     # 9 n-subtiles per core
SUBS = ((0, 512), (512, 512), (1024, 128))  # n-chunk psum subtiles
EXP_BIAS = -5.0

_compiled = {}


def _build(single=False):
    import concourse.bacc as bacc
    import concourse.bass as bass
    import concourse.mybir as mybir
    import concourse.tile as tile

    f16 = mybir.dt.float16
    f32 = mybir.dt.float32
    i16 = mybir.dt.int16
    AF = mybir.ActivationFunctionType
    ALU = mybir.AluOpType

    nc = bacc.Bacc("TRN2", target_bir_lowering=False, debug=False,
                   num_devices=1 if single else NCORES)

    xpad_io = nc.dram_tensor("xpad", [C, 98, 98], f16, kind="ExternalInput")
    x16_io = nc.dram_tensor("x16", [C, N], f16, kind="ExternalInput")
    w1_io = nc.dram_tensor("w1", [C, 9 * C], f16, kind="ExternalInput")
    w2_io = nc.dram_tensor("w2", [C, 9 * C], f16, kind="ExternalInput")
    w3_io = nc.dram_tensor("w3", [C, 9 * C], f16, kind="ExternalInput")
    tw_io = nc.dram_tensor("tw", [CI, C], f16, kind="ExternalInput")
    pw_io = nc.dram_tensor("pw", [CI, C], f16, kind="ExternalInput")
    gw_io = nc.dram_tensor("gw", [CI, C], f16, kind="ExternalInput")
    WwT_io = nc.dram_tensor("WwT", [CI, C], f16, kind="ExternalInput")
    ident_io = nc.dram_tensor("ident", [C, C], f32, kind="ExternalInput")
    xch_io = nc.dram_tensor("xch", [C, CH], f32, kind="ExternalInput")
    xch16_io = nc.dram_tensor("xch16", [C, CH], f16, kind="ExternalInput")
    out_io = nc.dram_tensor("out", [128, NSUB * C], f32, kind="ExternalOutput")

    with tile.TileContext(nc) as tc:
        with tc.tile_pool(name="persist", bufs=1) as pp, \
             tc.tile_pool(name="dram", bufs=1, space="DRAM") as dp:
            zsum = pp.tile([128, MT], f32, name="zsum")
            nb5 = pp.tile([128, 1], f32)
            nc.gpsimd.memset(nb5[:], EXP_BIAS)
            zinA = dp.tile([128, KA], f32)
            zoutA = dp.tile([128, KA], f32, addr_space="Shared")
            zinB = dp.tile([128, MT - KA], f32)
            zoutB = dp.tile([128, MT - KA], f32, addr_space="Shared")

            # cross-phase SBUF tensors
            NA = 24
            with tc.tile_pool(name="hand", bufs=1) as hp, \
                 tc.tile_pool(name="scA", bufs=1) as scpA, \
                 tc.tile_pool(name="p1ps", bufs=2, space="PSUM") as p1ps:
                v16 = hp.tile([C, CH], f16, name="v16")
                xg16 = hp.tile([C, N], f16, name="xg16")
                G16 = hp.tile([128, MT * C], f16, name="G16")
                G3 = G16[:].rearrange("p (j c) -> p j c", c=C)
                xgc = hp.tile([C, CH], f32, name="xgc")
                Q16 = hp.tile([C, C], f16)
                ET16 = hp.tile([C, C], f16)
                idsb = hp.tile([C, C], f32)
                zscr = hp.tile([128, CH], f16)
                aff16 = hp.tile([128, CH], f16)
                outT = hp.tile([128, NSUB * C], f32, name="outT")
                s_cacheA = scpA.tile([128, NA * CH], f16)

                def s_sl(j):
                    if j < NA:
                        return s_cacheA[:, j * CH:(j + 1) * CH]
                    r = MT - 1 - j
                    return s_cacheB[:, r * CH:(r + 1) * CH]

                def pass1_tile(j, _p1ps):
                    fps = _p1ps.tile([128, CH], f32, tag="fps", name="fps")
                    for o0, w in SUBS:
                        nc.tensor.matmul(fps[:, o0:o0 + w],
                                         xg16[:, j * 128:(j + 1) * 128],
                                         v16[:, o0:o0 + w],
                                         start=True, stop=True)
                    ssl = s_sl(j)
                    if j % 3 == 2:
                        # DVE path: Schraudolph bitcast exp (clamped at 0)
                        nc.vector.tensor_scalar(
                            aff16[:], fps[:], EXP_ALPHA, EXP_BETA,
                            ALU.mult, ALU.add)
                        nc.vector.tensor_scalar(
                            ssl.bitcast(i16), aff16[:], 0.0, 1.0,
                            ALU.max, ALU.mult)
                        nc.vector.tensor_scalar(
                            zscr[:], ssl, 1.0, 0.0, ALU.mult, ALU.add,
                            accum_out=zsum[:, j:j + 1])
                    else:
                        # ACT path: exact exp, Z for free via accum_out
                        nc.scalar.activation(ssl, fps[:], AF.Exp,
                                             bias=nb5[:], scale=1.0,
                                             accum_out=zsum[:, j:j + 1])

                # ==================== PHASE A ====================
                # pixel order is PERMUTED: n' = h*96 + r*12 + k maps to image
                # pixel (h, w=8k+r).  This makes every horizontal-upsample
                # write contiguous fp16 (DVE 4x mode); the host permutes
                # x16/xch inputs and de-permutes the output to match.
                with tc.tile_pool(name="mid", bufs=1) as pm:
                    yh = pm.tile([C, N], f16)
                    yh4 = yh[:].rearrange("c (h r k) -> c h r k", r=8, k=12)
                    yvp = pm.tile([C, 96, 14], f16)
                    dh = pm.tile([C, 96, 13], f16)
                    dhc = pm.tile([C, 12, 13], f16)
                    yvpc = pm.tile([C, 12, 14], f16)
                    x16sb = pm.tile([C, N], f16)
                    xchsb = pm.tile([C, CH], f32)
                    xch16sb = pm.tile([C, CH], f16)
                    twsb = pm.tile([CI, C], f16)
                    pwsb = pm.tile([CI, C], f16)
                    gwsb = pm.tile([CI, C], f16)
                    WwTsb = pm.tile([CI, C], f16)

                    # --- A1: convs + vertical upsample + own-row slices ---
                    with tc.tile_pool(name="pa1", bufs=1) as pa, \
                         tc.tile_pool(name="paps1", bufs=2, space="PSUM") as paps:
                        w1sb = pa.tile([C, 9 * C], f16)
                        nc.sync.dma_start(w1sb[:], w1_io[:])
                        w2sb = pa.tile([C, 9 * C], f16)
                        nc.sync.dma_start(w2sb[:], w2_io[:])
                        w3sb = pa.tile([C, 9 * C], f16)
                        nc.sync.dma_start(w3sb[:], w3_io[:])
                        xpad = pa.tile([C, 98, 98], f16)
                        for b in range(4):
                            r0, r1 = 26 * b, min(26 * b + 26, 98)
                            nc.sync.dma_start(xpad[:, r0:r1, :],
                                              xpad_io[:, r0:r1, :])
                        nc.sync.dma_start(twsb[:], tw_io[:])
                        nc.sync.dma_start(pwsb[:], pw_io[:])
                        nc.sync.dma_start(gwsb[:], gw_io[:])
                        nc.sync.dma_start(WwTsb[:], WwT_io[:])
                        nc.sync.dma_start(idsb[:], ident_io[:])
                        nc.sync.dma_start(xchsb[:], xch_io[:])
                        nc.sync.dma_start(xch16sb[:], xch16_io[:])
                        for b in range(2):
                            sl = slice(b * (N // 2), (b + 1) * (N // 2))
                            nc.sync.dma_start(x16sb[:, sl], x16_io[:, sl])

                        # conv1: 96x96 -> 48x48, stride 2, pad 1
                        # lrelu(x) = max(0.2*x, x) in one DVE op
                        y1p = pa.tile([C, 50, 50], f16)
                        nc.gpsimd.memset(y1p[:], 0.0)
                        for g in range(6):
                            ps1 = paps.tile([C, 8, 48], f32, tag="cv", name="ps1")
                            for t in range(9):
                                dy, dx = t // 3, t % 3
                                nc.tensor.matmul(
                                    ps1[:], w1sb[:, t * C:(t + 1) * C],
                                    xpad[:, 16 * g + dy: 16 * g + dy + 16: 2,
                                         dx: dx + 96: 2],
                                    start=(t == 0), stop=(t == 8))
                            sl1 = y1p[:, 1 + 8 * g: 9 + 8 * g, 1:49]
                            nc.vector.tensor_copy(sl1, ps1[:])
                            nc.vector.scalar_tensor_tensor(
                                sl1, sl1, 0.2, sl1, op0=ALU.mult, op1=ALU.max)

                        # conv2: 48x48 -> 24x24
                        y2p = pa.tile([C, 26, 26], f16)
                        nc.gpsimd.memset(y2p[:], 0.0)
                        for g in range(2):
                            ps2 = paps.tile([C, 12, 24], f32, tag="cv", name="ps2")
                            for t in range(9):
                                dy, dx = t // 3, t % 3
                                nc.tensor.matmul(
                                    ps2[:], w2sb[:, t * C:(t + 1) * C],
                                    y1p[:, 24 * g + dy: 24 * g + dy + 24: 2,
                                        dx: dx + 48: 2],
                                    start=(t == 0), stop=(t == 8))
                            sl2 = y2p[:, 1 + 12 * g: 13 + 12 * g, 1:25]
                            nc.vector.tensor_copy(sl2, ps2[:])
                            nc.vector.scalar_tensor_tensor(
                                sl2, sl2, 0.2, sl2, op0=ALU.mult, op1=ALU.max)

                        # conv3: 24x24 -> 12x12 (no activation)
                        ps3 = paps.tile([C, 12, 12], f32, tag="cv", name="ps3")
                        for t in range(9):
                            dy, dx = t // 3, t % 3
                            nc.tensor.matmul(
                                ps3[:], w3sb[:, t * C:(t + 1) * C],
                                y2p[:, dy: dy + 24: 2, dx: dx + 24: 2],
                                start=(t == 0), stop=(t == 8))
                        y3v = pa.tile([C, 14, 12], f32)
                        nc.vector.tensor_copy(y3v[:, 1:13, :], ps3[:])
                        nc.vector.tensor_copy(y3v[:, 0:1, :], ps3[:, 0:1, :])
                        nc.vector.tensor_copy(y3v[:, 13:14, :], ps3[:, 11:12, :])

                        # bilinear x8 vertical: out[8k+r] = X + b_r * (Y - X)
                        dv = pa.tile([C, 13, 12], f32)
                        nc.vector.tensor_sub(dv[:], y3v[:, 1:14, :], y3v[:, 0:13, :])
                        for r in range(8):
                            t = (r + 0.5) / 8 - 0.5
                            kr, b = (0, 1 + t) if r < 4 else (1, t)
                            nc.vector.scalar_tensor_tensor(
                                yvp[:, r:96:8, 1:13], dv[:, kr:kr + 12, :], float(b),
                                y3v[:, kr:kr + 12, :], op0=ALU.mult, op1=ALU.add)
                        nc.vector.tensor_copy(yvp[:, :, 0:1], yvp[:, :, 1:2])
                        nc.vector.tensor_copy(yvp[:, :, 13:14], yvp[:, :, 12:13])
                        nc.vector.tensor_sub(dh[:], yvp[:, :, 1:14], yvp[:, :, 0:13])

                        # own 12-row slices of yvp/dh so the v projection does
                        # not have to wait for the full horizontal upsample
                        with tc.tile_critical():
                            pid = nc.vector.partition_id()
                            dhf = dh[:].rearrange("c h w -> c (h w)")
                            yvf = yvp[:].rearrange("c h w -> c (h w)")
                            dcf = dhc[:].rearrange("c h w -> c (h w)")
                            ycf = yvpc[:].rearrange("c h w -> c (h w)")
                            nc.vector.tensor_copy(
                                dcf, dhf[:, bass.ds(pid * (12 * 13), 12 * 13)])
                            nc.vector.tensor_copy(
                                ycf, yvf[:, bass.ds(pid * (12 * 14), 12 * 14)])

                    # --- A2: gate, xg16, v projection, G^T (fp16 PE) ---
                    with tc.tile_pool(name="pa2", bufs=1) as pa, \
                         tc.tile_pool(name="paps2", bufs=1, space="PSUM") as paps:
                        # Q = Tw^T Pw, E^T = gw^T WwT  (both [64,64] fp16)
                        qps = paps.tile([C, C], f32, tag="g", name="qps",
                                        bufs=2)
                        nc.tensor.matmul(qps[:], twsb[:], pwsb[:],
                                         start=True, stop=True)
                        nc.vector.tensor_copy(Q16[:], qps[:])
                        eps = paps.tile([C, C], f32, tag="g", name="eps",
                                        bufs=2)
                        nc.tensor.matmul(eps[:], gwsb[:], WwTsb[:],
                                         start=True, stop=True)
                        nc.vector.tensor_copy(ET16[:], eps[:])

                        # own-chunk path: horizontal-upsample the own 12 rows
                        # (permuted order), sigmoid, v = Q^T xg_own (fp16)
                        yhc = pa.tile([C, CH], f16)
                        yhc4 = yhc[:].rearrange("c (h r k) -> c h r k",
                                                r=8, k=12)
                        for r in range(8):
                            t = (r + 0.5) / 8 - 0.5
                            kr, b = (0, 1 + t) if r < 4 else (1, t)
                            nc.vector.scalar_tensor_tensor(
                                yhc4[:, :, r, :], dhc[:, :, kr:kr + 12],
                                float(b), yvpc[:, :, kr:kr + 12],
                                op0=ALU.mult, op1=ALU.add)
                        gt16c = pa.tile([C, CH], f16)
                        nc.scalar.activation(gt16c[:], yhc[:], AF.Sigmoid)
                        xgc16 = pa.tile([C, CH], f16)
                        nc.vector.tensor_mul(xgc16[:], gt16c[:], xch16sb[:])
                        for o0, w in SUBS:
                            vps = paps.tile([C, 512], f32, tag="g", name="vps",
                                            bufs=2)
                            nc.tensor.matmul(vps[:, 0:w], Q16[:],
                                             xgc16[:, o0:o0 + w],
                                             start=True, stop=True)
                            nc.vector.tensor_copy(v16[:, o0:o0 + w], vps[:, 0:w])

                        # G^T m-tile groups: G16[m, c] = (E^T xg)^T, fp16
                        def gt_group(jj):
                            gps = paps.tile([128, 8 * C], f32, tag="g",
                                            name="gps", bufs=2)
                            for u in range(8):
                                j = jj + u
                                nc.tensor.matmul(gps[:, u * C:(u + 1) * C],
                                                 xg16[:, j * 128:(j + 1) * 128],
                                                 ET16[:], start=True, stop=True)
                            nc.vector.tensor_copy(
                                G16[:, jj * C:(jj + 8) * C], gps[:])

                        # gate pipeline: per 12-image-row chunk (1152 cols):
                        # horizontal upsample (DVE) -> sigmoid (ACT) -> xg16
                        # (DVE, contiguous fp16 4x writes in permuted order);
                        # sigmoids stay contiguous on ACT so the Exp act-table
                        # is not reloaded mid-cruise; the gate mul runs on
                        # Pool to keep DVE on upsample
                        for i in range(8):
                            sl = slice(i * CH, (i + 1) * CH)
                            rows = slice(12 * i, 12 * (i + 1))
                            for r in range(8):
                                t = (r + 0.5) / 8 - 0.5
                                kr, b = (0, 1 + t) if r < 4 else (1, t)
                                nc.vector.scalar_tensor_tensor(
                                    yh4[:, rows, r, :],
                                    dh[:, rows, kr:kr + 12], float(b),
                                    yvp[:, rows, kr:kr + 12],
                                    op0=ALU.mult, op1=ALU.add)
                            gt = pa.tile([C, CH], f16, tag="gt", name="gt",
                                         bufs=8)
                            nc.scalar.activation(gt[:], yh[:, sl], AF.Sigmoid)
                            nc.gpsimd.tensor_mul(xg16[:, sl], gt[:],
                                                 x16sb[:, sl])

                        nc.scalar.drain()

                        # xgc (fp32 residual for the pass-2 psum init)
                        nc.vector.tensor_mul(xgc[:], gt16c[:], xchsb[:])

                        # early pass-1 strips; G^T groups are deferred to
                        # strips 15..23 (by then every gate chunk is ready,
                        # so they never stall the in-order PE ahead of exps)
                        gtq = list(range(0, MT, 8))
                        for j in range(NA):
                            pass1_tile(j, p1ps)
                            if j >= 15 and gtq:
                                gt_group(gtq.pop(0))
                        while gtq:
                            gt_group(gtq.pop(0))

                # ============ PASS 1 with overlapped PASS 2 halves ==========
                with tc.tile_pool(name="scache", bufs=1) as scp, \
                     tc.tile_pool(name="p2ps", bufs=1, space="PSUM") as p2ps:
                    s_cacheB = scp.tile([128, (MT - NA) * CH], f16)
                    y2ps = p2ps.tile([128, NSUB * C], f32)

                    # init y2ps with xg^T so the residual rides the psum init.
                    # start=True pending-zeroes the whole 2KB psum bank, so
                    # only the first write into each bank may set it (slots
                    # 0-7 live in bank 0, slot 8 in bank 1).
                    for i in range(NSUB):
                        nc.tensor.matmul(y2ps[:, i * C:(i + 1) * C],
                                         xgc[:, i * 128:(i + 1) * 128],
                                         idsb[:], is_transpose=True,
                                         start=(i in (0, 8)), stop=False,
                                         skip_group_check=True)

                    def allreduce(zi, zo, jsl):
                        nc.sync.dma_start(zi[:], zsum[:, jsl])
                        if single:
                            nc.sync.dma_start(zo[:], zi[:])
                        else:
                            nc.gpsimd.collective_compute(
                                "AllReduce", ALU.add,
                                replica_groups=[list(range(NCORES))],
                                ins=[zi.opt()], outs=[zo.opt()])

                    def pass2_tile(j):
                        for i in range(NSUB):
                            nc.tensor.matmul(
                                y2ps[:, i * C:(i + 1) * C],
                                s_sl(j)[:, i * 128:(i + 1) * 128],
                                G16[:, j * C:(j + 1) * C],
                                start=False, stop=(j == MT - 1 and
                                                   i in (NSUB - 2, NSUB - 1)),
                                skip_group_check=True)

                    def scale_G(j0, cnt, zo, p2):
                        zf = p2.tile([128, KA], f32, tag="zf", name="zf",
                                     bufs=2)
                        nc.sync.dma_start(zf[:, 0:cnt], zo[:])
                        rz = p2.tile([128, KA], f32, tag="rz", name="rz",
                                     bufs=2)
                        nc.vector.reciprocal(rz[:, 0:cnt], zf[:, 0:cnt])
                        rzb = rz[:, 0:cnt].unsqueeze(-1).to_broadcast(
                            (128, cnt, C))
                        nc.vector.tensor_mul(G3[:, j0:j0 + cnt, :],
                                             G3[:, j0:j0 + cnt, :], rzb)

                    # asymmetric Z chunks: AR-A (48 tiles) fires ~30us before
                    # the exp cruise ends and lands just as it finishes; AR-B
                    # (24 tiles) is the only collective left on the tail.
                    with tc.tile_pool(name="p2", bufs=1) as p2:
                        for j in range(NA, KA):
                            pass1_tile(j, p1ps)
                        allreduce(zinA, zoutA, slice(0, KA))
                        for j in range(KA, MT):
                            pass1_tile(j, p1ps)
                        allreduce(zinB, zoutB, slice(KA, MT))
                        scale_G(0, KA, zoutA, p2)
                        for j in range(0, KA):
                            pass2_tile(j)
                        scale_G(KA, MT - KA, zoutB, p2)
                        for j in range(KA, MT):
                            pass2_tile(j)
                        nc.vector.tensor_copy(outT[:], y2ps[:])
                        nc.sync.dma_start(out_io[:], outT[:])

    nc.compile()
    return nc


def get_program():
    if "nc" not in _compiled:
        _compiled["nc"] = _build()
    return _compiled["nc"]


def _perm():
    # device pixel order n' = h*96 + r*12 + k  <->  image pixel (h, w=8k+r)
    h = np.arange(H)[:, None, None]
    r = np.arange(8)[None, :, None]
    k = np.arange(12)[None, None, :]
    return (h * 96 + 8 * k + r).reshape(-1)  # P[n'] = n


_P = _perm()


def make_in_maps(inputs):
    f16 = np.float16
    x = np.asarray(inputs["x"], np.float32).reshape(C, H, W)
    xflat = np.ascontiguousarray(x.reshape(C, N))
    xperm = np.ascontiguousarray(xflat[:, _P])
    xpad = np.zeros((C, 98, 98), f16)
    xpad[:, 1:97, 1:97] = x.astype(f16)

    def conv_w(w):
        # [o, i, dy, dx] -> [i, (dy dx), o]
        return np.ascontiguousarray(
            np.asarray(w, np.float32).transpose(1, 2, 3, 0).reshape(C, 9 * C)
        ).astype(f16)

    base = {
        "xpad": xpad,
        "x16": xperm.astype(f16),
        "w1": conv_w(inputs["d1_w"]),
        "w2": conv_w(inputs["d2_w"]),
        "w3": conv_w(inputs["d3_w"]),
        "tw": np.ascontiguousarray(
            np.asarray(inputs["th_w"], np.float32)[:, :, 0, 0]).astype(f16),
        "pw": np.ascontiguousarray(
            np.asarray(inputs["ph_w"], np.float32)[:, :, 0, 0]).astype(f16),
        "gw": np.ascontiguousarray(
            np.asarray(inputs["g_w"], np.float32)[:, :, 0, 0]).astype(f16),
        "WwT": np.ascontiguousarray(
            np.asarray(inputs["W_w"], np.float32)[:, :, 0, 0].T).astype(f16),
        "ident": np.eye(C, dtype=np.float32),
    }
    in_maps = []
    for k in range(NCORES):
        m = dict(base)
        xc = np.ascontiguousarray(xperm[:, k * CH:(k + 1) * CH])
        m["xch"] = xc
        m["xch16"] = xc.astype(f16)
        in_maps.append(m)
    return in_maps


def _detranspose(o):
    # out_io [128, 9*64] -> [64, 1152]: row p, sub i holds y2T[i*128+p, :]
    return np.ascontiguousarray(
        o.reshape(128, NSUB, C).transpose(1, 0, 2).reshape(CH, C).T)


def kernel(**inputs):
    from concourse import bass_utils

    nc = get_program()
    in_maps = make_in_maps(inputs)
    res = bass_utils.run_bass_kernel_spmd(nc, in_maps,
                                          core_ids=list(range(NCORES)))
    operm = np.concatenate([_detranspose(res.results[k]["out"])
                            for k in range(NCORES)], axis=1)
    out = np.empty_like(operm)
    out[:, _P] = operm
    return out.reshape(1, C, H, W).astype(np.float32)
